# revision 16
# baseline (speedup 1.0000x reference)
"""Multi-head attention (B=4, S=2048, D=1024, H=16) on 8 trn2 cores.

Sharding: core c -> batch b = c//2, head-half = c%2 (8 heads = 512 dims).
Each core computes attention for its (batch, 8 heads) and a partial output
projection over its 512 d-features; the host sums the two partials per batch
and adds the (bo + bv @ Wo.T) constant row vector.

v2: all matmul operands in bf16 (f32 PSUM accumulation) so LDWEIGHTS uses
fast-weight-load and the PE stays dense at full clock; x is converted once
and kept resident in SBUF; the attention pass software-pipelines
scores(u) / exp(u) / PV(u-1) / Q-proj(qc+1) / out-proj(qc-1) at 2-matmul
granularity so the Scalar engine's exp stream (the throughput floor) always
has PSUM tiles ready and the PE never idles long enough to trip the HAM
clock throttle.

Device dataflow (per core, all shapes hardcoded):
  Pass 1: K^T [128d(2 heads), S] (bf16, bias folded), V' per k-tile
          [128s, 8*(64+1)] (bf16, key-padding mask folded, +mask column for
          the softmax denominator). x converted to bf16 (resident), weights
          converted on the Scalar engine.
  Pass 2: per (q-chunk 512, head-pair): S^T[k,q] = K^T.T @ Q^T tiles ->
          ACT exp(x/8) -> P^T (bf16); [num^T; denom] = [V'|m].T @ P^T
          accumulated over k-tiles; reciprocal+broadcast+multiply ->
          valsT [d, s] (bf16). out[s,:] += valsT.T @ WoT per s-tile.
"""

import numpy as np
from contextlib import ExitStack

import concourse.bacc as bacc
import concourse.tile as tile
import concourse.mybir as mybir
from concourse.bass_utils import run_bass_kernel_spmd

F32 = mybir.dt.float32
BF16 = mybir.dt.bfloat16
EXP = mybir.ActivationFunctionType.Exp

S = 2048          # sequence length
D = 1024          # model dim
HD = 64           # head dim
NHL = 8           # heads per core
HP = 4            # head pairs per core (128 dims each)
DLOC = 512        # d-features per core
ET = D // 128     # 8 contraction tiles over D
ST = S // 128     # 16 s-tiles
QC = S // 512     # 4 query chunks of 512
KK = S // 128     # 16 key tiles of 128
VW = HD + 1       # V block width per head incl. mask column


def build_nc():
    nc = bacc.Bacc(None)
    xT = nc.dram_tensor("xT", [D, S], F32, kind="ExternalInput")
    wqT = nc.dram_tensor("wqT", [D, DLOC], F32, kind="ExternalInput")
    wkT = nc.dram_tensor("wkT", [D, DLOC], F32, kind="ExternalInput")
    wvT = nc.dram_tensor("wvT", [D, DLOC], F32, kind="ExternalInput")
    woT = nc.dram_tensor("woT", [DLOC, D], F32, kind="ExternalInput")
    bq = nc.dram_tensor("bq", [DLOC, 1], F32, kind="ExternalInput")
    bk = nc.dram_tensor("bk", [DLOC, 1], F32, kind="ExternalInput")
    msk = nc.dram_tensor("msk", [S, 1], F32, kind="ExternalInput")
    out = nc.dram_tensor("out", [S, D], F32, kind="ExternalOutput")

    with tile.TileContext(nc) as tc, ExitStack() as ctx:
        res = ctx.enter_context(tc.tile_pool(name="res", bufs=1))

        kt = [res.tile([128, S], BF16, tag=f"kt{i}", name=f"kt{i}") for i in range(HP)]
        vm = [res.tile([128, NHL * VW], BF16, tag=f"vm{i}", name=f"vm{i}") for i in range(KK)]
        valsT = [res.tile([128, S], BF16, tag=f"valsT{i}", name=f"valsT{i}") for i in range(HP)]
        xb = [res.tile([128, S], BF16, tag=f"xb{e}", name=f"xb{e}") for e in range(ET)]
        wq_sb = [res.tile([128, DLOC], BF16, tag=f"wq{e}", name=f"wq{e}") for e in range(ET)]
        wo_sb = [res.tile([128, D], BF16, tag=f"wo{i}", name=f"wo{i}") for i in range(HP)]

        m_sb = res.tile([128, ST], F32, tag="m_sb")
        nc.sync.dma_start(out=m_sb, in_=msk.rearrange("(a p) o -> p (a o)", p=128))
        bq_sb = res.tile([128, HP], F32, tag="bq_sb")
        nc.sync.dma_start(out=bq_sb, in_=bq.rearrange("(a p) o -> p (a o)", p=128))
        bk_sb = res.tile([128, HP], F32, tag="bk_sb")
        nc.sync.dma_start(out=bk_sb, in_=bk.rearrange("(a p) o -> p (a o)", p=128))
        ones8 = res.tile([128, NHL], BF16, tag="ones8")
        nc.vector.memset(ones8, 1.0)

        # Per-head Q^T tiles, zero-padded on the opposite 64 partitions so the
        # score matmuls contract over the full 128 partitions (the zero half
        # nulls the other head's K rows).  This keeps every matmul in the
        # kernel in the same 128-row PE mode - no mode-switch drains.
        # Two generations (even/odd q-chunk), managed explicitly.
        qtg = [[res.tile([128, 512], BF16, tag=f"qt{g}_{h}", name=f"qt{g}_{h}")
                for h in range(NHL)] for g in range(2)]
        for g in range(2):
            for h in range(NHL):
                zr = slice(64, 128) if h % 2 == 0 else slice(0, 64)
                nc.vector.memset(qtg[g][h][zr, :], 0.0)

        def qproj_bias(qt_gen, g, psQ):
            # psQ [128 (head-pair dims), 512] -> two per-head tiles
            nc.vector.tensor_scalar_add(qt_gen[2 * g][0:64, :], psQ[0:64, :],
                                        bq_sb[0:64, g:g + 1])
            nc.vector.tensor_scalar_add(qt_gen[2 * g + 1][64:128, :], psQ[64:128, :],
                                        bq_sb[64:128, g:g + 1])

        # ---------- Pass 1: V projection + K(head-pair 0) (stream xT once) ---
        # K for head-pairs 1-3 and the rest of the qc=0 Q projection are
        # emitted later as pipeline fillers inside the attention pass.
        wk_sb = [res.tile([128, DLOC], BF16, tag=f"wk{e}", name=f"wk{e}") for e in range(ET)]
        with tc.tile_pool(name="pw", bufs=1) as pw, \
             tc.tile_pool(name="wstg", bufs=4) as wstg, \
             tc.tile_pool(name="xstg", bufs=12) as xstg, \
             tc.tile_pool(name="psA", bufs=4, space="PSUM") as psA:
            wv_sb = [pw.tile([128, DLOC], BF16, tag=f"wv{e}", name=f"wv{e}") for e in range(ET)]
            for e in range(ET):
                stg = wstg.tile([128, DLOC], F32, tag="wstg", name=f"wkstg{e}")
                nc.sync.dma_start(out=stg, in_=wkT[e * 128:(e + 1) * 128, :])
                nc.scalar.copy(wk_sb[e], stg)
                stg2 = wstg.tile([128, DLOC], F32, tag="wstg", name=f"wvstg{e}")
                nc.sync.dma_start(out=stg2, in_=wvT[e * 128:(e + 1) * 128, :])
                nc.scalar.copy(wv_sb[e], stg2)

            def emit_xchunk(qc):
                cs_x = slice(qc * 512, (qc + 1) * 512)
                for e in range(ET):
                    xs = xstg.tile([128, 512], F32, tag="xstg", name=f"xs{qc}_{e}")
                    nc.sync.dma_start(out=xs, in_=xT[e * 128:(e + 1) * 128, cs_x])
                    nc.scalar.copy(xb[e][:, cs_x], xs)

            emit_xchunk(0)
            for qc in range(QC):
                cs = slice(qc * 512, (qc + 1) * 512)
                if qc + 1 < QC:
                    emit_xchunk(qc + 1)
                if qc == 1:
                    for e in range(ET):
                        stg = wstg.tile([128, DLOC], F32, tag="wstg", name=f"wqstg{e}")
                        nc.sync.dma_start(out=stg, in_=wqT[e * 128:(e + 1) * 128, :])
                        nc.vector.tensor_copy(wq_sb[e], stg)
                if qc == 2:
                    for i in range(HP):
                        stg = wstg.tile([128, D], F32, tag="wostg", name=f"wostg{i}")
                        nc.sync.dma_start(out=stg, in_=woT[i * 128:(i + 1) * 128, :])
                        nc.vector.tensor_copy(wo_sb[i], stg)
                psK = psA.tile([128, 512], F32, tag="psA", name=f"psK{qc}")
                for e in range(ET):
                    nc.tensor.matmul(psK, wk_sb[e][:, 0:128], xb[e][:, cs],
                                     start=(e == 0), stop=(e == ET - 1))
                nc.vector.tensor_scalar_add(kt[0][:, cs], psK, bk_sb[:, 0:1])
                for j in range(4):
                    # during the last chunk, compute Q(qc=0) for heads 0/1
                    if qc == QC - 1 and j == 0:
                        psQ0 = psA.tile([128, 512], F32, tag="psA", name="psQ00")
                        for e in range(ET):
                            nc.tensor.matmul(psQ0, wq_sb[e][:, 0:128],
                                             xb[e][:, 0:512],
                                             start=(e == 0), stop=(e == ET - 1))
                        qproj_bias(qtg[0], 0, psQ0)
                    st = qc * 4 + j
                    ss = slice(st * 128, (st + 1) * 128)
                    psV = psA.tile([128, 512], F32, tag="psA", name=f"psV{st}")
                    for e in range(ET):
                        nc.tensor.matmul(psV, xb[e][:, ss], wv_sb[e],
                                         start=(e == 0), stop=(e == ET - 1))
                    mc = m_sb[:, st:st + 1]
                    vmv = vm[st].rearrange("p (h w) -> p h w", w=VW)
                    psVv = psV.rearrange("p (h w) -> p h w", w=HD)
                    nc.vector.tensor_scalar_mul(vmv[:, :, 0:HD], psVv, mc)
                    nc.vector.tensor_scalar_mul(
                        vmv[:, :, HD:VW],
                        ones8.rearrange("p (h o) -> p h o", o=1), mc)

        # ---- Pass 2: pipelined attention + Q projection + out projection ----
        with tc.tile_pool(name="ptp", bufs=24) as ptp, \
             tc.tile_pool(name="sm", bufs=4) as sm, \
             tc.tile_pool(name="ob", bufs=2) as ob, \
             tc.tile_pool(name="psS", bufs=2, space="PSUM") as psSp, \
             tc.tile_pool(name="psO", bufs=2, space="PSUM") as psOp, \
             tc.tile_pool(name="psQ", bufs=1, space="PSUM") as psQp, \
             tc.tile_pool(name="psC", bufs=1, space="PSUM") as psCp:

            def make_qproj(qc_next, qt_gen):
                state = {}
                cs_n = slice(qc_next * 512, (qc_next + 1) * 512)

                def emit(it):
                    g, e = divmod(it, ET)
                    if e == 0:
                        state["psQ"] = psQp.tile([128, 512], F32, tag="psQ",
                                                 name=f"psQ{qc_next}_{g}")
                    nc.tensor.matmul(state["psQ"], wq_sb[e][:, g * 128:(g + 1) * 128],
                                     xb[e][:, cs_n], start=(e == 0), stop=(e == ET - 1))
                    if e == ET - 1:
                        qproj_bias(qt_gen, g, state["psQ"])
                return emit

            def make_outproj(qc_prev, pools):
                state = {}

                def emit(m):
                    grp, hp_i = divmod(m, HP)
                    stl, ec = divmod(grp, 2)
                    pool, ptag = pools[grp % len(pools)]
                    st = qc_prev * 4 + stl
                    ss = slice(st * 128, (st + 1) * 128)
                    es = slice(ec * 512, (ec + 1) * 512)
                    if hp_i == 0 and ec == 0:
                        state["ot"] = ob.tile([128, D], F32, tag="ot", name=f"ot{st}")
                    if hp_i == 0:
                        state["psC"] = pool.tile([128, 512], F32, tag=ptag,
                                                 name=f"psC{st}_{ec}")
                    nc.tensor.matmul(state["psC"], valsT[hp_i][:, ss],
                                     wo_sb[hp_i][:, es],
                                     start=(hp_i == 0), stop=(hp_i == HP - 1))
                    if hp_i == HP - 1:
                        nc.vector.tensor_copy(state["ot"][:, es], state["psC"])
                        if ec == 1:
                            nc.sync.dma_start(out=out[ss, :], in_=state["ot"])
                return emit

            # qc=0 filler queue: remaining K projections (head-pairs 1-3),
            # Q(0) head-pair groups 1-3, and the full Q(1) projection.
            # Each item is one 8-matmul accumulation group.
            fill_items = ([("qt0", 1)] + [("K", (1, 0)), ("K", (1, 1))] +
                          [("qt0", 2)] + [("K", (1, 2)), ("K", (1, 3))] +
                          [("qt0", 3)] +
                          [("K", (2, c)) for c in range(4)] +
                          [("K", (3, c)) for c in range(4)] +
                          [("qp1", g) for g in range(HP)])
            fill_state = {"idx": 0, "e": 0, "ps": None}

            def drain_fillers(n):
                for _ in range(n):
                    if fill_state["idx"] >= len(fill_items):
                        return
                    kind, arg = fill_items[fill_state["idx"]]
                    e = fill_state["e"]
                    if kind == "qt0":
                        g = arg
                        if e == 0:
                            fill_state["ps"] = psQp.tile([128, 512], F32, tag="psQ",
                                                         name=f"fq0_{g}")
                        nc.tensor.matmul(fill_state["ps"],
                                         wq_sb[e][:, g * 128:(g + 1) * 128],
                                         xb[e][:, 0:512],
                                         start=(e == 0), stop=(e == ET - 1))
                        if e == ET - 1:
                            qproj_bias(qtg[0], g, fill_state["ps"])
                    elif kind == "K":
                        hp_k, ck = arg
                        cs_k = slice(ck * 512, (ck + 1) * 512)
                        if e == 0:
                            fill_state["ps"] = psCp.tile([128, 512], F32, tag="psC",
                                                         name=f"fk{hp_k}_{ck}")
                        nc.tensor.matmul(fill_state["ps"],
                                         wk_sb[e][:, hp_k * 128:(hp_k + 1) * 128],
                                         xb[e][:, cs_k],
                                         start=(e == 0), stop=(e == ET - 1))
                        if e == ET - 1:
                            nc.vector.tensor_scalar_add(kt[hp_k][:, cs_k],
                                                        fill_state["ps"],
                                                        bk_sb[:, hp_k:hp_k + 1])
                    else:  # qp1: Q projection for qc=1
                        g = arg
                        if e == 0:
                            fill_state["ps"] = psQp.tile([128, 512], F32, tag="psQ",
                                                         name=f"fq1_{g}")
                        nc.tensor.matmul(fill_state["ps"],
                                         wq_sb[e][:, g * 128:(g + 1) * 128],
                                         xb[e][:, 512:1024],
                                         start=(e == 0), stop=(e == ET - 1))
                        if e == ET - 1:
                            qproj_bias(qtg[1], g, fill_state["ps"])
                    fill_state["e"] += 1
                    if fill_state["e"] == ET:
                        fill_state["e"] = 0
                        fill_state["idx"] += 1

            def emit_pv(unit, pts_u, psO_pair, kp):
                _, php = unit
                for h2 in range(2):
                    h_prev = php * 2 + h2
                    for u2 in range(2):
                        kk = 2 * kp + u2
                        nc.tensor.matmul(
                            psO_pair[h2][0:VW, :],
                            vm[kk][:, h_prev * VW:(h_prev + 1) * VW],
                            pts_u[h2][kp][:, u2 * 512:(u2 + 1) * 512],
                            start=(kk == 0), stop=(kk == KK - 1))

            def emit_norms(unit, psO_pair):
                uqc, uhp = unit
                ucs = slice(uqc * 512, (uqc + 1) * 512)
                for h2 in range(2):
                    hr = slice(h2 * 64, (h2 + 1) * 64)
                    dn = sm.tile([1, 512], F32, tag="dn", name=f"dn{uqc}_{uhp}_{h2}")
                    nc.vector.tensor_copy(dn, psO_pair[h2][HD:VW, :])
                    nc.vector.reciprocal_approx_fast(out=dn, in_=dn)
                    dnb = sm.tile([64, 512], F32, tag="dnb", name=f"dnb{uqc}_{uhp}_{h2}")
                    nc.gpsimd.partition_broadcast(dnb, dn)
                    nc.vector.tensor_mul(valsT[uhp][hr, ucs], psO_pair[h2][0:HD, :], dnb)

            qproj_emit = None
            outproj_emit = None
            qt_cur = None
            pts_prev = None
            prev_unit = None
            psO_prev = None

            for ui in range(QC * HP):
                qc, hp = divmod(ui, HP)
                if hp == 0:
                    qt_cur = qtg[qc % 2]
                    if 0 < qc < QC - 1:
                        # Q(1) comes from the qc=0 filler queue
                        qproj_emit = make_qproj(qc + 1, qtg[(qc + 1) % 2])
                    else:
                        qproj_emit = None
                    outproj_emit = (make_outproj(qc - 1, [(psCp, "psC")])
                                    if qc > 0 else None)

                pts_cur = [[None] * 8 for _ in range(2)]
                if prev_unit is not None:
                    psO_prev = [psOp.tile([128, 512], F32, tag="psO",
                                          name=f"psO{ui}_{h2}") for h2 in range(2)]
                if ui == QC * HP - 1:
                    # final unit: run its own PV in-loop (one iteration behind)
                    psO_self = [psQp.tile([128, 512], F32, tag="psQ", name="psOS0"),
                                psCp.tile([128, 512], F32, tag="psC", name="psOS1")]

                for kp in range(8):
                    it = hp * 8 + kp
                    psS_pair = [psSp.tile([128, 1024], F32, tag="psS",
                                          name=f"psS{ui}_{kp}_{h2}") for h2 in range(2)]
                    # score matmuls contract over the full 128 partitions;
                    # the zero half of the per-head Q tile nulls the other
                    # head's K rows
                    for u2 in range(2):
                        kk = 2 * kp + u2
                        ks = slice(kk * 128, (kk + 1) * 128)
                        for h2 in range(2):
                            nc.tensor.matmul(psS_pair[h2][:, u2 * 512:(u2 + 1) * 512],
                                             kt[hp][:, ks], qt_cur[hp * 2 + h2],
                                             start=True, stop=True)
                    for h2 in range(2):
                        pt = ptp.tile([128, 1024], BF16, tag="pt",
                                      name=f"pt{ui}_{kp}_{h2}")
                        nc.scalar.activation(pt, psS_pair[h2], EXP, scale=0.125)
                        pts_cur[h2][kp] = pt
                    # PV for the previous unit (one 128-contraction mode group)
                    if prev_unit is not None:
                        _, php = prev_unit
                        for h2 in range(2):
                            h_prev = php * 2 + h2
                            for u2 in range(2):
                                kk = 2 * kp + u2
                                nc.tensor.matmul(
                                    psO_prev[h2][0:VW, :],
                                    vm[kk][:, h_prev * VW:(h_prev + 1) * VW],
                                    pts_prev[h2][kp][:, u2 * 512:(u2 + 1) * 512],
                                    start=(kk == 0), stop=(kk == KK - 1))
                    # final unit: its own PV one iteration behind
                    if ui == QC * HP - 1 and kp > 0:
                        emit_pv((qc, hp), pts_cur, psO_self, kp - 1)
                    if qc == 0:
                        drain_fillers(6 if ui == 0 else 5)
                    if qproj_emit is not None:
                        qproj_emit(it)
                    if outproj_emit is not None and 8 <= it < 24:
                        m = (it - 8) * 2
                        outproj_emit(m)
                        outproj_emit(m + 1)
                    if kp == 7 and prev_unit is not None:
                        emit_norms(prev_unit, psO_prev)

                pts_prev = pts_cur
                prev_unit = (qc, hp)

            # epilogue: last PV step + norm for the final unit, then the
            # qc=3 out-projection double-banked across two PSUM pools
            emit_pv(prev_unit, pts_prev, psO_self, 7)
            emit_norms(prev_unit, psO_self)
            outproj_emit = make_outproj(QC - 1, [(psCp, "psC"), (psQp, "psQ")])
            for m in range(32):
                outproj_emit(m)

    nc.finalize()
    return nc


_NC_CACHE = None


def _get_nc():
    global _NC_CACHE
    if _NC_CACHE is None:
        _NC_CACHE = build_nc()
    return _NC_CACHE


def make_in_maps(x, mask, Wq, bq, Wk, bk, Wv, Wo):
    in_maps = []
    for c in range(8):
        b = c // 2
        dsl = slice((c % 2) * DLOC, (c % 2) * DLOC + DLOC)
        in_maps.append({
            "xT": np.ascontiguousarray(x[b].T, dtype=np.float32),
            "wqT": np.ascontiguousarray(Wq[dsl, :].T, dtype=np.float32),
            "wkT": np.ascontiguousarray(Wk[dsl, :].T, dtype=np.float32),
            "wvT": np.ascontiguousarray(Wv[dsl, :].T, dtype=np.float32),
            "woT": np.ascontiguousarray(Wo[:, dsl].T, dtype=np.float32),
            "bq": np.ascontiguousarray(bq[dsl], dtype=np.float32)[:, None],
            "bk": np.ascontiguousarray(bk[dsl], dtype=np.float32)[:, None],
            "msk": mask[b].astype(np.float32)[:, None],
        })
    return in_maps


def assemble(results, Wo, bo, bv):
    out = np.empty((4, S, D), dtype=np.float32)
    for b in range(4):
        out[b] = results[2 * b]["out"] + results[2 * b + 1]["out"]
    out += (bo + bv @ Wo.T).astype(np.float32)
    return out


def run(x, mask, Wq, bq, Wk, bk, Wv, bv, Wo, bo, trace=False):
    nc = _get_nc()
    in_maps = make_in_maps(x, mask, Wq, bq, Wk, bk, Wv, Wo)
    res = run_bass_kernel_spmd(nc, in_maps, list(range(8)), trace=trace)
    return assemble(res.results, Wo, bo, bv), res


def kernel(x, mask, Wq, bq, Wk, bk, Wv, bv, Wo, bo):
    out, _ = run(x, mask, Wq, bq, Wk, bk, Wv, bv, Wo, bo)
    return out


# revision 21
# speedup vs baseline: 1.0186x; 1.0186x over previous
"""Multi-head attention (B=4, S=2048, D=1024, H=16) on 8 trn2 cores.

Sharding: core c -> batch b = c//2, head-half = c%2 (8 heads = 512 dims).
Each core computes attention for its (batch, 8 heads) and a partial output
projection over its 512 d-features; the host sums the two partials per batch
and adds the (bo + bv @ Wo.T) constant row vector.

v2: all matmul operands in bf16 (f32 PSUM accumulation) so LDWEIGHTS uses
fast-weight-load and the PE stays dense at full clock; x is converted once
and kept resident in SBUF; the attention pass software-pipelines
scores(u) / exp(u) / PV(u-1) / Q-proj(qc+1) / out-proj(qc-1) at 2-matmul
granularity so the Scalar engine's exp stream (the throughput floor) always
has PSUM tiles ready and the PE never idles long enough to trip the HAM
clock throttle.

Device dataflow (per core, all shapes hardcoded):
  Pass 1: K^T [128d(2 heads), S] (bf16, bias folded), V' per k-tile
          [128s, 8*(64+1)] (bf16, key-padding mask folded, +mask column for
          the softmax denominator). x converted to bf16 (resident), weights
          converted on the Scalar engine.
  Pass 2: per (q-chunk 512, head-pair): S^T[k,q] = K^T.T @ Q^T tiles ->
          ACT exp(x/8) -> P^T (bf16); [num^T; denom] = [V'|m].T @ P^T
          accumulated over k-tiles; reciprocal+broadcast+multiply ->
          valsT [d, s] (bf16). out[s,:] += valsT.T @ WoT per s-tile.
"""

import numpy as np
from contextlib import ExitStack

import concourse.bacc as bacc
import concourse.tile as tile
import concourse.mybir as mybir
from concourse.bass_utils import run_bass_kernel_spmd

F32 = mybir.dt.float32
BF16 = mybir.dt.bfloat16
EXP = mybir.ActivationFunctionType.Exp

S = 2048          # sequence length
D = 1024          # model dim
HD = 64           # head dim
NHL = 8           # heads per core
HP = 4            # head pairs per core (128 dims each)
DLOC = 512        # d-features per core
ET = D // 128     # 8 contraction tiles over D
ST = S // 128     # 16 s-tiles
QC = S // 512     # 4 query chunks of 512
KK = S // 128     # 16 key tiles of 128
VW = HD + 1       # V block width per head incl. mask column


def build_nc():
    nc = bacc.Bacc(None)
    xT = nc.dram_tensor("xT", [D, S], F32, kind="ExternalInput")
    wqT = nc.dram_tensor("wqT", [D, DLOC], F32, kind="ExternalInput")
    wkT = nc.dram_tensor("wkT", [D, DLOC], F32, kind="ExternalInput")
    wvT = nc.dram_tensor("wvT", [D, DLOC], F32, kind="ExternalInput")
    woT = nc.dram_tensor("woT", [DLOC, D], F32, kind="ExternalInput")
    bq = nc.dram_tensor("bq", [DLOC, 1], F32, kind="ExternalInput")
    bk = nc.dram_tensor("bk", [DLOC, 1], F32, kind="ExternalInput")
    msk = nc.dram_tensor("msk", [S, 1], F32, kind="ExternalInput")
    out = nc.dram_tensor("out", [S, D], F32, kind="ExternalOutput")

    with tile.TileContext(nc) as tc, ExitStack() as ctx:
        res = ctx.enter_context(tc.tile_pool(name="res", bufs=1))

        kt = [res.tile([128, S], BF16, tag=f"kt{i}", name=f"kt{i}") for i in range(HP)]
        vm = [res.tile([128, NHL * VW], BF16, tag=f"vm{i}", name=f"vm{i}") for i in range(KK)]
        valsT = [res.tile([128, S], BF16, tag=f"valsT{i}", name=f"valsT{i}") for i in range(HP)]
        xb = [res.tile([128, S], BF16, tag=f"xb{e}", name=f"xb{e}") for e in range(ET)]
        wq_sb = [res.tile([128, DLOC], BF16, tag=f"wq{e}", name=f"wq{e}") for e in range(ET)]
        wo_sb = [res.tile([128, D], BF16, tag=f"wo{i}", name=f"wo{i}") for i in range(HP)]

        m_sb = res.tile([128, ST], F32, tag="m_sb")
        nc.sync.dma_start(out=m_sb, in_=msk.rearrange("(a p) o -> p (a o)", p=128))
        bq_sb = res.tile([128, HP], F32, tag="bq_sb")
        nc.sync.dma_start(out=bq_sb, in_=bq.rearrange("(a p) o -> p (a o)", p=128))
        bk_sb = res.tile([128, HP], F32, tag="bk_sb")
        nc.sync.dma_start(out=bk_sb, in_=bk.rearrange("(a p) o -> p (a o)", p=128))
        ones8 = res.tile([128, NHL], BF16, tag="ones8")
        nc.vector.memset(ones8, 1.0)

        # Per-head Q^T tiles, zero-padded on the opposite 64 partitions so the
        # score matmuls contract over the full 128 partitions (the zero half
        # nulls the other head's K rows).  This keeps every matmul in the
        # kernel in the same 128-row PE mode - no mode-switch drains.
        # Two generations (even/odd q-chunk), managed explicitly.
        qtg = [[res.tile([128, 512], BF16, tag=f"qt{g}_{h}", name=f"qt{g}_{h}")
                for h in range(NHL)] for g in range(2)]
        for g in range(2):
            for h in range(NHL):
                zr = slice(64, 128) if h % 2 == 0 else slice(0, 64)
                nc.vector.memset(qtg[g][h][zr, :], 0.0)

        def qproj_bias(qt_gen, g, psQ):
            # psQ [128 (head-pair dims), 512] -> two per-head tiles
            nc.vector.tensor_scalar_add(qt_gen[2 * g][0:64, :], psQ[0:64, :],
                                        bq_sb[0:64, g:g + 1])
            nc.vector.tensor_scalar_add(qt_gen[2 * g + 1][64:128, :], psQ[64:128, :],
                                        bq_sb[64:128, g:g + 1])

        # ---------- Pass 1: K and V projections (stream xT once) ----------
        with tc.tile_pool(name="pw", bufs=1) as pw, \
             tc.tile_pool(name="wstg", bufs=8) as wstg, \
             tc.tile_pool(name="xstg", bufs=14) as xstg, \
             tc.tile_pool(name="psA", bufs=4, space="PSUM") as psA:
            wk_sb = [pw.tile([128, DLOC], BF16, tag=f"wk{e}", name=f"wk{e}") for e in range(ET)]
            wv_sb = [pw.tile([128, DLOC], BF16, tag=f"wv{e}", name=f"wv{e}") for e in range(ET)]
            # wk first on the scalar queue (K projection unblocks first),
            # wv copies deferred until after chunk 0 of x
            wkstg = []
            for e in range(ET):
                stg = wstg.tile([128, DLOC], F32, tag="wstg", name=f"wkstg{e}")
                nc.sync.dma_start(out=stg, in_=wkT[e * 128:(e + 1) * 128, :])
                nc.scalar.copy(wk_sb[e], stg)
            wvstg = []
            for e in range(ET):
                stg = wstg.tile([128, DLOC], F32, tag="wstg", name=f"wvstg{e}")
                nc.sync.dma_start(out=stg, in_=wvT[e * 128:(e + 1) * 128, :])
                wvstg.append(stg)

            def emit_xchunk(qc):
                cs_x = slice(qc * 512, (qc + 1) * 512)
                for e in range(ET):
                    xs = xstg.tile([128, 512], F32, tag="xstg", name=f"xs{qc}_{e}")
                    nc.sync.dma_start(out=xs, in_=xT[e * 128:(e + 1) * 128, cs_x])
                    nc.scalar.copy(xb[e][:, cs_x], xs)

            emit_xchunk(0)
            for e in range(ET):
                nc.scalar.copy(wv_sb[e], wvstg[e])
            for qc in range(QC):
                cs = slice(qc * 512, (qc + 1) * 512)
                if qc + 1 < QC:
                    emit_xchunk(qc + 1)
                if qc == 1:
                    for e in range(ET):
                        stg = wstg.tile([128, DLOC], F32, tag="wstg", name=f"wqstg{e}")
                        nc.sync.dma_start(out=stg, in_=wqT[e * 128:(e + 1) * 128, :])
                        nc.vector.tensor_copy(wq_sb[e], stg)
                if qc == 2:
                    for i in range(HP):
                        stg = wstg.tile([128, D], F32, tag="wostg", name=f"wostg{i}")
                        nc.sync.dma_start(out=stg, in_=woT[i * 128:(i + 1) * 128, :])
                        nc.vector.tensor_copy(wo_sb[i], stg)
                for hp in range(HP):
                    hcols = slice(hp * 128, (hp + 1) * 128)
                    psK = psA.tile([128, 512], F32, tag="psA", name=f"psK{qc}_{hp}")
                    for e in range(ET):
                        nc.tensor.matmul(psK, wk_sb[e][:, hcols], xb[e][:, cs],
                                         start=(e == 0), stop=(e == ET - 1))
                    nc.vector.tensor_scalar_add(kt[hp][:, cs], psK, bk_sb[:, hp:hp + 1])
                for j in range(4):
                    # during the last chunk, interleave the qc=0 Q projection
                    if qc == QC - 1:
                        psQ0 = psA.tile([128, 512], F32, tag="psA", name=f"psQ0_{j}")
                        for e in range(ET):
                            nc.tensor.matmul(psQ0, wq_sb[e][:, j * 128:(j + 1) * 128],
                                             xb[e][:, 0:512],
                                             start=(e == 0), stop=(e == ET - 1))
                        qproj_bias(qtg[0], j, psQ0)
                    st = qc * 4 + j
                    ss = slice(st * 128, (st + 1) * 128)
                    psV = psA.tile([128, 512], F32, tag="psA", name=f"psV{st}")
                    for e in range(ET):
                        nc.tensor.matmul(psV, xb[e][:, ss], wv_sb[e],
                                         start=(e == 0), stop=(e == ET - 1))
                    mc = m_sb[:, st:st + 1]
                    vmv = vm[st].rearrange("p (h w) -> p h w", w=VW)
                    psVv = psV.rearrange("p (h w) -> p h w", w=HD)
                    nc.vector.tensor_scalar_mul(vmv[:, :, 0:HD], psVv, mc)
                    nc.vector.tensor_scalar_mul(
                        vmv[:, :, HD:VW],
                        ones8.rearrange("p (h o) -> p h o", o=1), mc)

        # ---- Pass 2: pipelined attention + Q projection + out projection ----
        with tc.tile_pool(name="ptp", bufs=24) as ptp, \
             tc.tile_pool(name="sm", bufs=4) as sm, \
             tc.tile_pool(name="ob", bufs=2) as ob, \
             tc.tile_pool(name="psS", bufs=2, space="PSUM") as psSp, \
             tc.tile_pool(name="psO", bufs=2, space="PSUM") as psOp, \
             tc.tile_pool(name="psQ", bufs=1, space="PSUM") as psQp, \
             tc.tile_pool(name="psC", bufs=1, space="PSUM") as psCp:

            def make_qproj(qc_next, qt_gen):
                state = {}
                cs_n = slice(qc_next * 512, (qc_next + 1) * 512)

                def emit(it):
                    g, e = divmod(it, ET)
                    if e == 0:
                        state["psQ"] = psQp.tile([128, 512], F32, tag="psQ",
                                                 name=f"psQ{qc_next}_{g}")
                    nc.tensor.matmul(state["psQ"], wq_sb[e][:, g * 128:(g + 1) * 128],
                                     xb[e][:, cs_n], start=(e == 0), stop=(e == ET - 1))
                    if e == ET - 1:
                        qproj_bias(qt_gen, g, state["psQ"])
                return emit

            def make_outproj(qc_prev, pools):
                state = {}

                def emit(m):
                    grp, hp_i = divmod(m, HP)
                    stl, ec = divmod(grp, 2)
                    pool, ptag = pools[grp % len(pools)]
                    st = qc_prev * 4 + stl
                    ss = slice(st * 128, (st + 1) * 128)
                    es = slice(ec * 512, (ec + 1) * 512)
                    if hp_i == 0 and ec == 0:
                        state["ot"] = ob.tile([128, D], F32, tag="ot", name=f"ot{st}")
                    if hp_i == 0:
                        state["psC"] = pool.tile([128, 512], F32, tag=ptag,
                                                 name=f"psC{st}_{ec}")
                    nc.tensor.matmul(state["psC"], valsT[hp_i][:, ss],
                                     wo_sb[hp_i][:, es],
                                     start=(hp_i == 0), stop=(hp_i == HP - 1))
                    if hp_i == HP - 1:
                        nc.vector.tensor_copy(state["ot"][:, es], state["psC"])
                        if ec == 1:
                            nc.sync.dma_start(out=out[ss, :], in_=state["ot"])
                return emit



            def emit_pv(unit, pts_u, psO_pair, kp):
                _, php = unit
                for h2 in range(2):
                    h_prev = php * 2 + h2
                    for u2 in range(2):
                        kk = 2 * kp + u2
                        nc.tensor.matmul(
                            psO_pair[h2][0:VW, :],
                            vm[kk][:, h_prev * VW:(h_prev + 1) * VW],
                            pts_u[h2][kp][:, u2 * 512:(u2 + 1) * 512],
                            start=(kk == 0), stop=(kk == KK - 1))

            def emit_norms(unit, psO_pair):
                uqc, uhp = unit
                ucs = slice(uqc * 512, (uqc + 1) * 512)
                for h2 in range(2):
                    hr = slice(h2 * 64, (h2 + 1) * 64)
                    dn = sm.tile([1, 512], F32, tag="dn", name=f"dn{uqc}_{uhp}_{h2}")
                    nc.vector.tensor_copy(dn, psO_pair[h2][HD:VW, :])
                    nc.vector.reciprocal_approx_fast(out=dn, in_=dn)
                    dnb = sm.tile([64, 512], F32, tag="dnb", name=f"dnb{uqc}_{uhp}_{h2}")
                    nc.gpsimd.partition_broadcast(dnb, dn)
                    nc.vector.tensor_mul(valsT[uhp][hr, ucs], psO_pair[h2][0:HD, :], dnb)

            qproj_emit = None
            outproj_emit = None
            qt_cur = None
            pts_prev = None
            prev_unit = None
            psO_prev = None

            for ui in range(QC * HP):
                qc, hp = divmod(ui, HP)
                if hp == 0:
                    qt_cur = qtg[qc % 2]
                    if qc + 1 < QC:
                        qproj_emit = make_qproj(qc + 1, qtg[(qc + 1) % 2])
                    else:
                        qproj_emit = None
                    outproj_emit = (make_outproj(qc - 1, [(psCp, "psC")])
                                    if qc > 0 else None)

                pts_cur = [[None] * 8 for _ in range(2)]
                if prev_unit is not None:
                    psO_prev = [psOp.tile([128, 512], F32, tag="psO",
                                          name=f"psO{ui}_{h2}") for h2 in range(2)]
                if ui == QC * HP - 1:
                    # final unit: run its own PV in-loop (one iteration behind)
                    psO_self = [psQp.tile([128, 512], F32, tag="psQ", name="psOS0"),
                                psCp.tile([128, 512], F32, tag="psC", name="psOS1")]

                for kp in range(8):
                    it = hp * 8 + kp
                    psS_pair = [psSp.tile([128, 1024], F32, tag="psS",
                                          name=f"psS{ui}_{kp}_{h2}") for h2 in range(2)]
                    # PV for the previous unit first: hides the psS WAR wait
                    # (this iteration's scores reuse the slots the previous
                    # iteration's exps are still draining)
                    if prev_unit is not None:
                        _, php = prev_unit
                        for h2 in range(2):
                            h_prev = php * 2 + h2
                            for u2 in range(2):
                                kk = 2 * kp + u2
                                nc.tensor.matmul(
                                    psO_prev[h2][0:VW, :],
                                    vm[kk][:, h_prev * VW:(h_prev + 1) * VW],
                                    pts_prev[h2][kp][:, u2 * 512:(u2 + 1) * 512],
                                    start=(kk == 0), stop=(kk == KK - 1))
                    # final unit: its own PV one iteration behind
                    if ui == QC * HP - 1 and kp > 0:
                        emit_pv((qc, hp), pts_cur, psO_self, kp - 1)
                    # score matmuls contract over the full 128 partitions;
                    # the zero half of the per-head Q tile nulls the other
                    # head's K rows
                    for u2 in range(2):
                        kk = 2 * kp + u2
                        ks = slice(kk * 128, (kk + 1) * 128)
                        for h2 in range(2):
                            nc.tensor.matmul(psS_pair[h2][:, u2 * 512:(u2 + 1) * 512],
                                             kt[hp][:, ks], qt_cur[hp * 2 + h2],
                                             start=True, stop=True)
                    for h2 in range(2):
                        pt = ptp.tile([128, 1024], BF16, tag="pt",
                                      name=f"pt{ui}_{kp}_{h2}")
                        nc.scalar.activation(pt, psS_pair[h2], EXP, scale=0.125)
                        pts_cur[h2][kp] = pt
                    if qproj_emit is not None:
                        qproj_emit(it)
                    if outproj_emit is not None and 8 <= it < 24:
                        m = (it - 8) * 2
                        outproj_emit(m)
                        outproj_emit(m + 1)
                    if kp == 7 and prev_unit is not None:
                        emit_norms(prev_unit, psO_prev)

                pts_prev = pts_cur
                prev_unit = (qc, hp)

            # epilogue: last PV step + norm for the final unit, then the
            # qc=3 out-projection double-banked across two PSUM pools
            emit_pv(prev_unit, pts_prev, psO_self, 7)
            emit_norms(prev_unit, psO_self)
            outproj_emit = make_outproj(QC - 1, [(psCp, "psC"), (psQp, "psQ")])
            for m in range(32):
                outproj_emit(m)

    nc.finalize()
    return nc


_NC_CACHE = None


def _get_nc():
    global _NC_CACHE
    if _NC_CACHE is None:
        _NC_CACHE = build_nc()
    return _NC_CACHE


def make_in_maps(x, mask, Wq, bq, Wk, bk, Wv, Wo):
    in_maps = []
    for c in range(8):
        b = c // 2
        dsl = slice((c % 2) * DLOC, (c % 2) * DLOC + DLOC)
        in_maps.append({
            "xT": np.ascontiguousarray(x[b].T, dtype=np.float32),
            "wqT": np.ascontiguousarray(Wq[dsl, :].T, dtype=np.float32),
            "wkT": np.ascontiguousarray(Wk[dsl, :].T, dtype=np.float32),
            "wvT": np.ascontiguousarray(Wv[dsl, :].T, dtype=np.float32),
            "woT": np.ascontiguousarray(Wo[:, dsl].T, dtype=np.float32),
            "bq": np.ascontiguousarray(bq[dsl], dtype=np.float32)[:, None],
            "bk": np.ascontiguousarray(bk[dsl], dtype=np.float32)[:, None],
            "msk": mask[b].astype(np.float32)[:, None],
        })
    return in_maps


def assemble(results, Wo, bo, bv):
    out = np.empty((4, S, D), dtype=np.float32)
    for b in range(4):
        out[b] = results[2 * b]["out"] + results[2 * b + 1]["out"]
    out += (bo + bv @ Wo.T).astype(np.float32)
    return out


def run(x, mask, Wq, bq, Wk, bk, Wv, bv, Wo, bo, trace=False):
    nc = _get_nc()
    in_maps = make_in_maps(x, mask, Wq, bq, Wk, bk, Wv, Wo)
    res = run_bass_kernel_spmd(nc, in_maps, list(range(8)), trace=trace)
    return assemble(res.results, Wo, bo, bv), res


def kernel(x, mask, Wq, bq, Wk, bk, Wv, bv, Wo, bo):
    out, _ = run(x, mask, Wq, bq, Wk, bk, Wv, bv, Wo, bo)
    return out


# revision 26
# speedup vs baseline: 1.2019x; 1.1800x over previous
"""Multi-head attention (B=4, S=2048, D=1024, H=16) on 8 trn2 cores.

Sharding: core c -> batch b = c//2, head-half = c%2 (8 heads = 512 dims).
Each core computes attention for its (batch, 8 heads) and a partial output
projection over its 512 d-features; the host sums the two partials per batch
and adds the (bo + bv @ Wo.T) constant row vector.

v2: all matmul operands in bf16 (f32 PSUM accumulation) so LDWEIGHTS uses
fast-weight-load and the PE stays dense at full clock; x is converted once
and kept resident in SBUF; the attention pass software-pipelines
scores(u) / exp(u) / PV(u-1) / Q-proj(qc+1) / out-proj(qc-1) at 2-matmul
granularity so the Scalar engine's exp stream (the throughput floor) always
has PSUM tiles ready and the PE never idles long enough to trip the HAM
clock throttle.

Device dataflow (per core, all shapes hardcoded):
  Pass 1: K^T [128d(2 heads), S] (bf16, bias folded), V' per k-tile
          [128s, 8*(64+1)] (bf16, key-padding mask folded, +mask column for
          the softmax denominator). x converted to bf16 (resident), weights
          converted on the Scalar engine.
  Pass 2: per (q-chunk 512, head-pair): S^T[k,q] = K^T.T @ Q^T tiles ->
          ACT exp(x/8) -> P^T (bf16); [num^T; denom] = [V'|m].T @ P^T
          accumulated over k-tiles; reciprocal+broadcast+multiply ->
          valsT [d, s] (bf16). out[s,:] += valsT.T @ WoT per s-tile.
"""

import numpy as np
from contextlib import ExitStack

import concourse.bacc as bacc
import concourse.tile as tile
import concourse.mybir as mybir
from concourse.bass_utils import run_bass_kernel_spmd

F32 = mybir.dt.float32
BF16 = mybir.dt.bfloat16
EXP = mybir.ActivationFunctionType.Exp

S = 2048          # sequence length
D = 1024          # model dim
HD = 64           # head dim
NHL = 8           # heads per core
HP = 4            # head pairs per core (128 dims each)
DLOC = 512        # d-features per core
ET = D // 128     # 8 contraction tiles over D
ST = S // 128     # 16 s-tiles
QC = S // 512     # 4 query chunks of 512
KK = S // 128     # 16 key tiles of 128
VW = HD + 1       # V block width per head incl. mask column


def build_nc():
    nc = bacc.Bacc(None)
    xT = nc.dram_tensor("xT", [D, S], F32, kind="ExternalInput")
    wqT = nc.dram_tensor("wqT", [D, DLOC], F32, kind="ExternalInput")
    wkT = nc.dram_tensor("wkT", [D, DLOC], F32, kind="ExternalInput")
    wvT = nc.dram_tensor("wvT", [D, DLOC], F32, kind="ExternalInput")
    woT = nc.dram_tensor("woT", [DLOC, D], F32, kind="ExternalInput")
    bq = nc.dram_tensor("bq", [DLOC, 1], F32, kind="ExternalInput")
    bk = nc.dram_tensor("bk", [DLOC, 1], F32, kind="ExternalInput")
    msk = nc.dram_tensor("msk", [S, 1], F32, kind="ExternalInput")
    out = nc.dram_tensor("out", [S, D], F32, kind="ExternalOutput")

    with tile.TileContext(nc) as tc, ExitStack() as ctx:
        res = ctx.enter_context(tc.tile_pool(name="res", bufs=1))

        kt = [res.tile([128, S], BF16, tag=f"kt{i}", name=f"kt{i}") for i in range(HP)]
        vm = [res.tile([128, NHL * VW], BF16, tag=f"vm{i}", name=f"vm{i}") for i in range(KK)]
        valsT = [res.tile([128, S], BF16, tag=f"valsT{i}", name=f"valsT{i}") for i in range(HP)]
        xb = [res.tile([128, S], BF16, tag=f"xb{e}", name=f"xb{e}") for e in range(ET)]
        wq_sb = [res.tile([128, DLOC], BF16, tag=f"wq{e}", name=f"wq{e}") for e in range(ET)]
        wo_sb = [res.tile([128, D], BF16, tag=f"wo{i}", name=f"wo{i}") for i in range(HP)]

        m_sb = res.tile([128, ST], F32, tag="m_sb")
        nc.sync.dma_start(out=m_sb, in_=msk.rearrange("(a p) o -> p (a o)", p=128))
        bq_sb = res.tile([128, HP], F32, tag="bq_sb")
        nc.sync.dma_start(out=bq_sb, in_=bq.rearrange("(a p) o -> p (a o)", p=128))
        bk_sb = res.tile([128, HP], F32, tag="bk_sb")
        nc.sync.dma_start(out=bk_sb, in_=bk.rearrange("(a p) o -> p (a o)", p=128))
        ones8 = res.tile([128, NHL], BF16, tag="ones8")
        nc.vector.memset(ones8, 1.0)

        def emit_vm(st, psV):
            mc = m_sb[:, st:st + 1]
            vmv = vm[st].rearrange("p (h w) -> p h w", w=VW)
            psVv = psV.rearrange("p (h w) -> p h w", w=HD)
            nc.vector.tensor_scalar_mul(vmv[:, :, 0:HD], psVv, mc)
            nc.vector.tensor_scalar_mul(
                vmv[:, :, HD:VW], ones8.rearrange("p (h o) -> p h o", o=1), mc)

        # Per-head Q^T tiles, zero-padded on the opposite 64 partitions so the
        # score matmuls contract over the full 128 partitions (the zero half
        # nulls the other head's K rows).  This keeps every matmul in the
        # kernel in the same 128-row PE mode - no mode-switch drains.
        # Two generations (even/odd q-chunk), managed explicitly.
        qtg = [[res.tile([128, 512], BF16, tag=f"qt{g}_{h}", name=f"qt{g}_{h}")
                for h in range(NHL)] for g in range(2)]
        for g in range(2):
            for h in range(NHL):
                zr = slice(64, 128) if h % 2 == 0 else slice(0, 64)
                nc.vector.memset(qtg[g][h][zr, :], 0.0)

        def qproj_bias(qt_gen, g, psQ):
            # psQ [128 (head-pair dims), 512] -> two per-head tiles
            nc.vector.tensor_scalar_add(qt_gen[2 * g][0:64, :], psQ[0:64, :],
                                        bq_sb[0:64, g:g + 1])
            nc.vector.tensor_scalar_add(qt_gen[2 * g + 1][64:128, :], psQ[64:128, :],
                                        bq_sb[64:128, g:g + 1])

        # ---------- Pass 1: K and V projections (stream xT once) ----------
        # wv stays resident: the last four V-projection groups run inside
        # unit 0 of the attention pass
        wv_sb = [res.tile([128, DLOC], BF16, tag=f"wv{e}", name=f"wv{e}") for e in range(ET)]
        with tc.tile_pool(name="pw", bufs=1) as pw, \
             tc.tile_pool(name="wstg", bufs=8) as wstg, \
             tc.tile_pool(name="xstg", bufs=14) as xstg, \
             tc.tile_pool(name="psA", bufs=4, space="PSUM") as psA:
            wk_sb = [pw.tile([128, DLOC], BF16, tag=f"wk{e}", name=f"wk{e}") for e in range(ET)]
            # wk first on the scalar queue (K projection unblocks first),
            # wv copies deferred until after chunk 0 of x
            wkstg = []
            for e in range(ET):
                stg = wstg.tile([128, DLOC], F32, tag="wstg", name=f"wkstg{e}")
                nc.sync.dma_start(out=stg, in_=wkT[e * 128:(e + 1) * 128, :])
                nc.scalar.copy(wk_sb[e], stg)
            wvstg = []
            for e in range(ET):
                stg = wstg.tile([128, DLOC], F32, tag="wstg", name=f"wvstg{e}")
                nc.sync.dma_start(out=stg, in_=wvT[e * 128:(e + 1) * 128, :])
                wvstg.append(stg)

            def emit_xchunk(qc):
                cs_x = slice(qc * 512, (qc + 1) * 512)
                for e in range(ET):
                    xs = xstg.tile([128, 512], F32, tag="xstg", name=f"xs{qc}_{e}")
                    nc.sync.dma_start(out=xs, in_=xT[e * 128:(e + 1) * 128, cs_x])
                    nc.scalar.copy(xb[e][:, cs_x], xs)

            emit_xchunk(0)
            for e in range(ET):
                nc.scalar.copy(wv_sb[e], wvstg[e])
            for qc in range(QC):
                cs = slice(qc * 512, (qc + 1) * 512)
                if qc + 1 < QC:
                    emit_xchunk(qc + 1)
                if qc == 1:
                    for e in range(ET):
                        stg = wstg.tile([128, DLOC], F32, tag="wstg", name=f"wqstg{e}")
                        nc.sync.dma_start(out=stg, in_=wqT[e * 128:(e + 1) * 128, :])
                        nc.vector.tensor_copy(wq_sb[e], stg)
                if qc == 2:
                    for i in range(HP):
                        stg = wstg.tile([128, D], F32, tag="wostg", name=f"wostg{i}")
                        nc.sync.dma_start(out=stg, in_=woT[i * 128:(i + 1) * 128, :])
                        nc.vector.tensor_copy(wo_sb[i], stg)
                for hp in range(HP):
                    hcols = slice(hp * 128, (hp + 1) * 128)
                    psK = psA.tile([128, 512], F32, tag="psA", name=f"psK{qc}_{hp}")
                    for e in range(ET):
                        nc.tensor.matmul(psK, wk_sb[e][:, hcols], xb[e][:, cs],
                                         start=(e == 0), stop=(e == ET - 1))
                    nc.vector.tensor_scalar_add(kt[hp][:, cs], psK, bk_sb[:, hp:hp + 1])
                for j in range(4):
                    # last chunk: the qc=0 Q projection replaces the V groups
                    # (s-tiles 12-15 are computed as unit-0 fillers instead)
                    if qc == QC - 1:
                        psQ0 = psA.tile([128, 512], F32, tag="psA", name=f"psQ0_{j}")
                        for e in range(ET):
                            nc.tensor.matmul(psQ0, wq_sb[e][:, j * 128:(j + 1) * 128],
                                             xb[e][:, 0:512],
                                             start=(e == 0), stop=(e == ET - 1))
                        qproj_bias(qtg[0], j, psQ0)
                        continue
                    st = qc * 4 + j
                    ss = slice(st * 128, (st + 1) * 128)
                    psV = psA.tile([128, 512], F32, tag="psA", name=f"psV{st}")
                    for e in range(ET):
                        nc.tensor.matmul(psV, xb[e][:, ss], wv_sb[e],
                                         start=(e == 0), stop=(e == ET - 1))
                    emit_vm(st, psV)

        # ---- Pass 2: pipelined attention + Q projection + out projection ----
        with tc.tile_pool(name="ptp", bufs=24) as ptp, \
             tc.tile_pool(name="sm", bufs=4) as sm, \
             tc.tile_pool(name="ob", bufs=2) as ob, \
             tc.tile_pool(name="psS", bufs=2, space="PSUM") as psSp, \
             tc.tile_pool(name="psO", bufs=2, space="PSUM") as psOp, \
             tc.tile_pool(name="psQ", bufs=1, space="PSUM") as psQp, \
             tc.tile_pool(name="psC", bufs=1, space="PSUM") as psCp:

            def make_qproj(qc_next, qt_gen):
                state = {}
                cs_n = slice(qc_next * 512, (qc_next + 1) * 512)

                def emit(it):
                    g, e = divmod(it, ET)
                    if e == 0:
                        state["psQ"] = psQp.tile([128, 512], F32, tag="psQ",
                                                 name=f"psQ{qc_next}_{g}")
                    nc.tensor.matmul(state["psQ"], wq_sb[e][:, g * 128:(g + 1) * 128],
                                     xb[e][:, cs_n], start=(e == 0), stop=(e == ET - 1))
                    if e == ET - 1:
                        qproj_bias(qt_gen, g, state["psQ"])
                return emit

            def make_outproj(qc_prev, pools):
                state = {}

                def emit(m):
                    grp, hp_i = divmod(m, HP)
                    stl, ec = divmod(grp, 2)
                    pool, ptag = pools[grp % len(pools)]
                    st = qc_prev * 4 + stl
                    ss = slice(st * 128, (st + 1) * 128)
                    es = slice(ec * 512, (ec + 1) * 512)
                    if hp_i == 0 and ec == 0:
                        state["ot"] = ob.tile([128, D], F32, tag="ot", name=f"ot{st}")
                    if hp_i == 0:
                        state["psC"] = pool.tile([128, 512], F32, tag=ptag,
                                                 name=f"psC{st}_{ec}")
                    nc.tensor.matmul(state["psC"], valsT[hp_i][:, ss],
                                     wo_sb[hp_i][:, es],
                                     start=(hp_i == 0), stop=(hp_i == HP - 1))
                    if hp_i == HP - 1:
                        nc.vector.tensor_copy(state["ot"][:, es], state["psC"])
                        if ec == 1:
                            nc.sync.dma_start(out=out[ss, :], in_=state["ot"])
                return emit



            def emit_pv(unit, pts_u, psO_pair, kp):
                _, php = unit
                for h2 in range(2):
                    h_prev = php * 2 + h2
                    for u2 in range(2):
                        kk = 2 * kp + u2
                        nc.tensor.matmul(
                            psO_pair[h2][0:VW, :],
                            vm[kk][:, h_prev * VW:(h_prev + 1) * VW],
                            pts_u[h2][kp][:, u2 * 512:(u2 + 1) * 512],
                            start=(kk == 0), stop=(kk == KK - 1))

            def emit_norms(unit, psO_pair):
                uqc, uhp = unit
                ucs = slice(uqc * 512, (uqc + 1) * 512)
                for h2 in range(2):
                    hr = slice(h2 * 64, (h2 + 1) * 64)
                    dn = sm.tile([1, 512], F32, tag="dn", name=f"dn{uqc}_{uhp}_{h2}")
                    nc.vector.tensor_copy(dn, psO_pair[h2][HD:VW, :])
                    nc.vector.reciprocal_approx_fast(out=dn, in_=dn)
                    dnb = sm.tile([64, 512], F32, tag="dnb", name=f"dnb{uqc}_{uhp}_{h2}")
                    nc.gpsimd.partition_broadcast(dnb, dn)
                    nc.vector.tensor_mul(valsT[uhp][hr, ucs], psO_pair[h2][0:HD, :], dnb)

            qproj_emit = None
            outproj_emit = None
            qt_cur = None
            pts_prev = None
            prev_unit = None
            psO_prev = None

            for ui in range(QC * HP):
                qc, hp = divmod(ui, HP)
                if hp == 0:
                    qt_cur = qtg[qc % 2]
                    if qc + 1 < QC:
                        qproj_emit = make_qproj(qc + 1, qtg[(qc + 1) % 2])
                    else:
                        qproj_emit = None
                    outproj_emit = (make_outproj(qc - 1, [(psCp, "psC")])
                                    if qc > 0 else None)

                pts_cur = [[None] * 8 for _ in range(2)]
                if prev_unit is not None:
                    psO_prev = [psOp.tile([128, 512], F32, tag="psO",
                                          name=f"psO{ui}_{h2}") for h2 in range(2)]
                if ui == QC * HP - 1:
                    # final unit: run its own PV in-loop (one iteration behind)
                    psO_self = [psQp.tile([128, 512], F32, tag="psQ", name="psOS0"),
                                psCp.tile([128, 512], F32, tag="psC", name="psOS1")]

                for kp in range(8):
                    it = hp * 8 + kp
                    psS_pair = [psSp.tile([128, 1024], F32, tag="psS",
                                          name=f"psS{ui}_{kp}_{h2}") for h2 in range(2)]
                    # PV for the previous unit first: hides the psS WAR wait
                    # (this iteration's scores reuse the slots the previous
                    # iteration's exps are still draining)
                    if prev_unit is not None:
                        _, php = prev_unit
                        for h2 in range(2):
                            h_prev = php * 2 + h2
                            for u2 in range(2):
                                kk = 2 * kp + u2
                                nc.tensor.matmul(
                                    psO_prev[h2][0:VW, :],
                                    vm[kk][:, h_prev * VW:(h_prev + 1) * VW],
                                    pts_prev[h2][kp][:, u2 * 512:(u2 + 1) * 512],
                                    start=(kk == 0), stop=(kk == KK - 1))
                    # final unit: its own PV one iteration behind
                    if ui == QC * HP - 1 and kp > 0:
                        emit_pv((qc, hp), pts_cur, psO_self, kp - 1)
                    # score matmuls contract over the full 128 partitions;
                    # the zero half of the per-head Q tile nulls the other
                    # head's K rows
                    for u2 in range(2):
                        kk = 2 * kp + u2
                        ks = slice(kk * 128, (kk + 1) * 128)
                        for h2 in range(2):
                            nc.tensor.matmul(psS_pair[h2][:, u2 * 512:(u2 + 1) * 512],
                                             kt[hp][:, ks], qt_cur[hp * 2 + h2],
                                             start=True, stop=True)
                    for h2 in range(2):
                        pt = ptp.tile([128, 1024], BF16, tag="pt",
                                      name=f"pt{ui}_{kp}_{h2}")
                        nc.scalar.activation(pt, psS_pair[h2], EXP, scale=0.125)
                        pts_cur[h2][kp] = pt
                    # unit 0: finish V projection s-tiles 12-15 (needed by
                    # PV(u0) only late in unit 1) to pace PE with the exps
                    if ui == 0:
                        g, half = divmod(kp, 2)
                        st_v = 12 + g
                        ss_v = slice(st_v * 128, (st_v + 1) * 128)
                        if half == 0:
                            vfill_ps = psCp.tile([128, 512], F32, tag="psC",
                                                 name=f"fv{st_v}")
                        for m in range(4):
                            e = half * 4 + m
                            nc.tensor.matmul(vfill_ps, xb[e][:, ss_v], wv_sb[e],
                                             start=(e == 0), stop=(e == ET - 1))
                        if half == 1:
                            emit_vm(st_v, vfill_ps)
                    if qproj_emit is not None:
                        qproj_emit(it)
                    if outproj_emit is not None and 8 <= it < 24:
                        m = (it - 8) * 2
                        outproj_emit(m)
                        outproj_emit(m + 1)
                    if kp == 7 and prev_unit is not None:
                        emit_norms(prev_unit, psO_prev)

                pts_prev = pts_cur
                prev_unit = (qc, hp)

            # epilogue: last PV step + norm for the final unit; the qc=3
            # out-projection spreads over four PSUM slots, pre-running the
            # head-pair 0-2 partials while the final norms drain so the PE
            # stays busy (and at high p-state)
            emit_pv(prev_unit, pts_prev, psO_self, 7)
            emit_norms(prev_unit, psO_self)
            ep_pools = [(psCp, "psC"), (psQp, "psQ"), (psOp, "psO"), (psOp, "psO")]
            ep_ps = {}
            ep_ot = {}

            def ep_mm(grp, hp_i):
                stl, ec = divmod(grp, 2)
                st = (QC - 1) * 4 + stl
                ss = slice(st * 128, (st + 1) * 128)
                es = slice(ec * 512, (ec + 1) * 512)
                if hp_i == 0:
                    pool, ptag = ep_pools[grp % 4]
                    ep_ps[grp] = pool.tile([128, 512], F32, tag=ptag,
                                           name=f"epC{grp}")
                    if ec == 0:
                        ep_ot[stl] = ob.tile([128, D], F32, tag="ot", name=f"ot{st}")
                nc.tensor.matmul(ep_ps[grp], valsT[hp_i][:, ss], wo_sb[hp_i][:, es],
                                 start=(hp_i == 0), stop=(hp_i == HP - 1))
                if hp_i == HP - 1:
                    nc.vector.tensor_copy(ep_ot[stl][:, es], ep_ps[grp])
                    if ec == 1:
                        nc.sync.dma_start(out=out[ss, :], in_=ep_ot[stl])

            for hp_i in range(3):
                for grp in range(4):
                    ep_mm(grp, hp_i)
            for grp in range(4):
                ep_mm(grp, 3)
            for grp in range(4, 8):
                for hp_i in range(HP):
                    ep_mm(grp, hp_i)

    nc.finalize()
    return nc


_NC_CACHE = None


def _get_nc():
    global _NC_CACHE
    if _NC_CACHE is None:
        _NC_CACHE = build_nc()
    return _NC_CACHE


def make_in_maps(x, mask, Wq, bq, Wk, bk, Wv, Wo):
    in_maps = []
    for c in range(8):
        b = c // 2
        dsl = slice((c % 2) * DLOC, (c % 2) * DLOC + DLOC)
        in_maps.append({
            "xT": np.ascontiguousarray(x[b].T, dtype=np.float32),
            "wqT": np.ascontiguousarray(Wq[dsl, :].T, dtype=np.float32),
            "wkT": np.ascontiguousarray(Wk[dsl, :].T, dtype=np.float32),
            "wvT": np.ascontiguousarray(Wv[dsl, :].T, dtype=np.float32),
            "woT": np.ascontiguousarray(Wo[:, dsl].T, dtype=np.float32),
            "bq": np.ascontiguousarray(bq[dsl], dtype=np.float32)[:, None],
            "bk": np.ascontiguousarray(bk[dsl], dtype=np.float32)[:, None],
            "msk": mask[b].astype(np.float32)[:, None],
        })
    return in_maps


def assemble(results, Wo, bo, bv):
    out = np.empty((4, S, D), dtype=np.float32)
    for b in range(4):
        out[b] = results[2 * b]["out"] + results[2 * b + 1]["out"]
    out += (bo + bv @ Wo.T).astype(np.float32)
    return out


def run(x, mask, Wq, bq, Wk, bk, Wv, bv, Wo, bo, trace=False):
    nc = _get_nc()
    in_maps = make_in_maps(x, mask, Wq, bq, Wk, bk, Wv, Wo)
    res = run_bass_kernel_spmd(nc, in_maps, list(range(8)), trace=trace)
    return assemble(res.results, Wo, bo, bv), res


def kernel(x, mask, Wq, bq, Wk, bk, Wv, bv, Wo, bo):
    out, _ = run(x, mask, Wq, bq, Wk, bk, Wv, bv, Wo, bo)
    return out


# revision 28
# speedup vs baseline: 1.2317x; 1.0247x over previous
"""Multi-head attention (B=4, S=2048, D=1024, H=16) on 8 trn2 cores.

Sharding: core c -> batch b = c//2, head-half = c%2 (8 heads = 512 dims).
Each core computes attention for its (batch, 8 heads) and a partial output
projection over its 512 d-features; the host sums the two partials per batch
and adds the (bo + bv @ Wo.T) constant row vector.

Key optimizations (603us baseline -> ~410us):
- All matmul operands bf16 (f32 PSUM accumulation): fast weight load,
  dense PE issue at full clock (no HAM throttle), halved SBUF footprint.
  x is converted once on the Scalar engine and kept resident.
- Every matmul contracts over the full 128 partitions: the per-head Q^T
  tiles are zero-padded on the opposite head's 64 partitions, so the score
  matmuls avoid the ~100ns PE mode-switch drain that 64-contraction
  matmuls (mixed with 128-contraction ones) would pay.
- The attention pass software-pipelines scores(u) / exp(u) / PV(u-1) /
  Q-proj(qc+1) / out-proj(qc-1) at 2-4 matmul granularity, keeping the
  Scalar engine's exp stream (the throughput floor, ~270us busy) and the
  PE (~350us busy) both >90% utilized. PV is emitted before the scores of
  each iteration to hide the psS slot WAR on the previous exps.
- Unit 0 (no PV yet) additionally finishes the last four V-projection
  groups; the final unit runs its own PV one iteration behind the exps;
  the closing out-projection pre-runs head-pair 0-2 partials across four
  PSUM slots while the final softmax norms drain.

Device dataflow (per core, all shapes hardcoded):
  Pass 1: K^T [128d(2 heads), S] (bf16, bias folded), V' per k-tile
          [128s, 8*(64+1)] (bf16, key-padding mask folded, +mask column for
          the softmax denominator).
  Pass 2: per (q-chunk 512, head-pair): S^T[k,q] = K^T.T @ Q^T tiles ->
          ACT exp(x/8) -> P^T (bf16); [num^T; denom] = [V'|m].T @ P^T
          accumulated over k-tiles; reciprocal+broadcast+multiply ->
          valsT [d, s] (bf16). out[s,:] += valsT.T @ WoT per s-tile.
"""

import numpy as np
from contextlib import ExitStack

import concourse.bacc as bacc
import concourse.tile as tile
import concourse.mybir as mybir
from concourse.bass_utils import run_bass_kernel_spmd

F32 = mybir.dt.float32
BF16 = mybir.dt.bfloat16
EXP = mybir.ActivationFunctionType.Exp

S = 2048          # sequence length
D = 1024          # model dim
HD = 64           # head dim
NHL = 8           # heads per core
HP = 4            # head pairs per core (128 dims each)
DLOC = 512        # d-features per core
ET = D // 128     # 8 contraction tiles over D
ST = S // 128     # 16 s-tiles
QC = S // 512     # 4 query chunks of 512
KK = S // 128     # 16 key tiles of 128
VW = HD + 1       # V block width per head incl. mask column


def build_nc():
    nc = bacc.Bacc(None)
    xT = nc.dram_tensor("xT", [D, S], F32, kind="ExternalInput")
    wqT = nc.dram_tensor("wqT", [D, DLOC], F32, kind="ExternalInput")
    wkT = nc.dram_tensor("wkT", [D, DLOC], F32, kind="ExternalInput")
    wvT = nc.dram_tensor("wvT", [D, DLOC], F32, kind="ExternalInput")
    woT = nc.dram_tensor("woT", [DLOC, D], F32, kind="ExternalInput")
    bq = nc.dram_tensor("bq", [DLOC, 1], F32, kind="ExternalInput")
    bk = nc.dram_tensor("bk", [DLOC, 1], F32, kind="ExternalInput")
    msk = nc.dram_tensor("msk", [S, 1], F32, kind="ExternalInput")
    out = nc.dram_tensor("out", [S, D], F32, kind="ExternalOutput")

    with tile.TileContext(nc) as tc, ExitStack() as ctx:
        res = ctx.enter_context(tc.tile_pool(name="res", bufs=1))

        kt = [res.tile([128, S], BF16, tag=f"kt{i}", name=f"kt{i}") for i in range(HP)]
        vm = [res.tile([128, NHL * VW], BF16, tag=f"vm{i}", name=f"vm{i}") for i in range(KK)]
        valsT = [res.tile([128, S], BF16, tag=f"valsT{i}", name=f"valsT{i}") for i in range(HP)]
        xb = [res.tile([128, S], BF16, tag=f"xb{e}", name=f"xb{e}") for e in range(ET)]
        wq_sb = [res.tile([128, DLOC], BF16, tag=f"wq{e}", name=f"wq{e}") for e in range(ET)]
        wo_sb = [res.tile([128, D], BF16, tag=f"wo{i}", name=f"wo{i}") for i in range(HP)]

        m_sb = res.tile([128, ST], F32, tag="m_sb")
        nc.sync.dma_start(out=m_sb, in_=msk.rearrange("(a p) o -> p (a o)", p=128))
        bq_sb = res.tile([128, HP], F32, tag="bq_sb")
        nc.sync.dma_start(out=bq_sb, in_=bq.rearrange("(a p) o -> p (a o)", p=128))
        bk_sb = res.tile([128, HP], F32, tag="bk_sb")
        nc.sync.dma_start(out=bk_sb, in_=bk.rearrange("(a p) o -> p (a o)", p=128))
        ones8 = res.tile([128, NHL], BF16, tag="ones8")
        nc.vector.memset(ones8, 1.0)
        # warm up the GPSIMD ucode library early: the first
        # partition_broadcast otherwise pays a ~7us cold-start inside the
        # first softmax-normalization chain, stalling the PE pipeline
        gwarm = res.tile([64, ST], F32, tag="gwarm")
        nc.gpsimd.partition_broadcast(gwarm, m_sb[0:1, :])

        def emit_vm(st, psV):
            mc = m_sb[:, st:st + 1]
            vmv = vm[st].rearrange("p (h w) -> p h w", w=VW)
            psVv = psV.rearrange("p (h w) -> p h w", w=HD)
            nc.vector.tensor_scalar_mul(vmv[:, :, 0:HD], psVv, mc)
            nc.vector.tensor_scalar_mul(
                vmv[:, :, HD:VW], ones8.rearrange("p (h o) -> p h o", o=1), mc)

        # Per-head Q^T tiles, zero-padded on the opposite 64 partitions so the
        # score matmuls contract over the full 128 partitions (the zero half
        # nulls the other head's K rows).  This keeps every matmul in the
        # kernel in the same 128-row PE mode - no mode-switch drains.
        # Two generations (even/odd q-chunk), managed explicitly.
        qtg = [[res.tile([128, 512], BF16, tag=f"qt{g}_{h}", name=f"qt{g}_{h}")
                for h in range(NHL)] for g in range(2)]
        for g in range(2):
            for h in range(NHL):
                zr = slice(64, 128) if h % 2 == 0 else slice(0, 64)
                nc.vector.memset(qtg[g][h][zr, :], 0.0)

        def qproj_bias(qt_gen, g, psQ):
            # psQ [128 (head-pair dims), 512] -> two per-head tiles
            nc.vector.tensor_scalar_add(qt_gen[2 * g][0:64, :], psQ[0:64, :],
                                        bq_sb[0:64, g:g + 1])
            nc.vector.tensor_scalar_add(qt_gen[2 * g + 1][64:128, :], psQ[64:128, :],
                                        bq_sb[64:128, g:g + 1])

        # ---------- Pass 1: K and V projections (stream xT once) ----------
        # wv stays resident: the last four V-projection groups run inside
        # unit 0 of the attention pass
        wv_sb = [res.tile([128, DLOC], BF16, tag=f"wv{e}", name=f"wv{e}") for e in range(ET)]
        with tc.tile_pool(name="pw", bufs=1) as pw, \
             tc.tile_pool(name="wstg", bufs=8) as wstg, \
             tc.tile_pool(name="xstg", bufs=14) as xstg, \
             tc.tile_pool(name="psA", bufs=4, space="PSUM") as psA:
            wk_sb = [pw.tile([128, DLOC], BF16, tag=f"wk{e}", name=f"wk{e}") for e in range(ET)]
            # wk first on the scalar queue (K projection unblocks first),
            # wv copies deferred until after chunk 0 of x
            wkstg = []
            for e in range(ET):
                stg = wstg.tile([128, DLOC], F32, tag="wstg", name=f"wkstg{e}")
                nc.sync.dma_start(out=stg, in_=wkT[e * 128:(e + 1) * 128, :])
                nc.scalar.copy(wk_sb[e], stg)
            wvstg = []
            for e in range(ET):
                stg = wstg.tile([128, DLOC], F32, tag="wstg", name=f"wvstg{e}")
                nc.sync.dma_start(out=stg, in_=wvT[e * 128:(e + 1) * 128, :])
                wvstg.append(stg)

            def emit_xchunk(qc):
                cs_x = slice(qc * 512, (qc + 1) * 512)
                for e in range(ET):
                    xs = xstg.tile([128, 512], F32, tag="xstg", name=f"xs{qc}_{e}")
                    nc.sync.dma_start(out=xs, in_=xT[e * 128:(e + 1) * 128, cs_x])
                    nc.scalar.copy(xb[e][:, cs_x], xs)

            emit_xchunk(0)
            for e in range(ET):
                nc.scalar.copy(wv_sb[e], wvstg[e])
            for qc in range(QC):
                cs = slice(qc * 512, (qc + 1) * 512)
                if qc + 1 < QC:
                    emit_xchunk(qc + 1)
                if qc == 1:
                    for e in range(ET):
                        stg = wstg.tile([128, DLOC], F32, tag="wstg", name=f"wqstg{e}")
                        nc.sync.dma_start(out=stg, in_=wqT[e * 128:(e + 1) * 128, :])
                        nc.vector.tensor_copy(wq_sb[e], stg)
                if qc == 2:
                    for i in range(HP):
                        stg = wstg.tile([128, D], F32, tag="wostg", name=f"wostg{i}")
                        nc.sync.dma_start(out=stg, in_=woT[i * 128:(i + 1) * 128, :])
                        nc.vector.tensor_copy(wo_sb[i], stg)
                for hp in range(HP):
                    hcols = slice(hp * 128, (hp + 1) * 128)
                    psK = psA.tile([128, 512], F32, tag="psA", name=f"psK{qc}_{hp}")
                    for e in range(ET):
                        nc.tensor.matmul(psK, wk_sb[e][:, hcols], xb[e][:, cs],
                                         start=(e == 0), stop=(e == ET - 1))
                    nc.vector.tensor_scalar_add(kt[hp][:, cs], psK, bk_sb[:, hp:hp + 1])
                for j in range(4):
                    # last chunk: the qc=0 Q projection replaces the V groups
                    # (s-tiles 12-15 are computed as unit-0 fillers instead)
                    if qc == QC - 1:
                        psQ0 = psA.tile([128, 512], F32, tag="psA", name=f"psQ0_{j}")
                        for e in range(ET):
                            nc.tensor.matmul(psQ0, wq_sb[e][:, j * 128:(j + 1) * 128],
                                             xb[e][:, 0:512],
                                             start=(e == 0), stop=(e == ET - 1))
                        qproj_bias(qtg[0], j, psQ0)
                        continue
                    st = qc * 4 + j
                    ss = slice(st * 128, (st + 1) * 128)
                    psV = psA.tile([128, 512], F32, tag="psA", name=f"psV{st}")
                    for e in range(ET):
                        nc.tensor.matmul(psV, xb[e][:, ss], wv_sb[e],
                                         start=(e == 0), stop=(e == ET - 1))
                    emit_vm(st, psV)

        # ---- Pass 2: pipelined attention + Q projection + out projection ----
        with tc.tile_pool(name="ptp", bufs=24) as ptp, \
             tc.tile_pool(name="sm", bufs=4) as sm, \
             tc.tile_pool(name="ob", bufs=2) as ob, \
             tc.tile_pool(name="psS", bufs=2, space="PSUM") as psSp, \
             tc.tile_pool(name="psO", bufs=2, space="PSUM") as psOp, \
             tc.tile_pool(name="psQ", bufs=1, space="PSUM") as psQp, \
             tc.tile_pool(name="psC", bufs=1, space="PSUM") as psCp:

            def make_qproj(qc_next, qt_gen):
                state = {}
                cs_n = slice(qc_next * 512, (qc_next + 1) * 512)

                def emit(it):
                    g, e = divmod(it, ET)
                    if e == 0:
                        state["psQ"] = psQp.tile([128, 512], F32, tag="psQ",
                                                 name=f"psQ{qc_next}_{g}")
                    nc.tensor.matmul(state["psQ"], wq_sb[e][:, g * 128:(g + 1) * 128],
                                     xb[e][:, cs_n], start=(e == 0), stop=(e == ET - 1))
                    if e == ET - 1:
                        qproj_bias(qt_gen, g, state["psQ"])
                return emit

            def make_outproj(qc_prev, pools):
                state = {}

                def emit(m):
                    grp, hp_i = divmod(m, HP)
                    stl, ec = divmod(grp, 2)
                    pool, ptag = pools[grp % len(pools)]
                    st = qc_prev * 4 + stl
                    ss = slice(st * 128, (st + 1) * 128)
                    es = slice(ec * 512, (ec + 1) * 512)
                    if hp_i == 0 and ec == 0:
                        state["ot"] = ob.tile([128, D], F32, tag="ot", name=f"ot{st}")
                    if hp_i == 0:
                        state["psC"] = pool.tile([128, 512], F32, tag=ptag,
                                                 name=f"psC{st}_{ec}")
                    nc.tensor.matmul(state["psC"], valsT[hp_i][:, ss],
                                     wo_sb[hp_i][:, es],
                                     start=(hp_i == 0), stop=(hp_i == HP - 1))
                    if hp_i == HP - 1:
                        nc.vector.tensor_copy(state["ot"][:, es], state["psC"])
                        if ec == 1:
                            nc.sync.dma_start(out=out[ss, :], in_=state["ot"])
                return emit



            def emit_pv(unit, pts_u, psO_pair, kp):
                _, php = unit
                for h2 in range(2):
                    h_prev = php * 2 + h2
                    for u2 in range(2):
                        kk = 2 * kp + u2
                        nc.tensor.matmul(
                            psO_pair[h2][0:VW, :],
                            vm[kk][:, h_prev * VW:(h_prev + 1) * VW],
                            pts_u[h2][kp][:, u2 * 512:(u2 + 1) * 512],
                            start=(kk == 0), stop=(kk == KK - 1))

            def emit_norms(unit, psO_pair):
                uqc, uhp = unit
                ucs = slice(uqc * 512, (uqc + 1) * 512)
                for h2 in range(2):
                    hr = slice(h2 * 64, (h2 + 1) * 64)
                    dn = sm.tile([1, 512], F32, tag="dn", name=f"dn{uqc}_{uhp}_{h2}")
                    nc.vector.tensor_copy(dn, psO_pair[h2][HD:VW, :])
                    nc.vector.reciprocal_approx_fast(out=dn, in_=dn)
                    dnb = sm.tile([64, 512], F32, tag="dnb", name=f"dnb{uqc}_{uhp}_{h2}")
                    nc.gpsimd.partition_broadcast(dnb, dn)
                    nc.vector.tensor_mul(valsT[uhp][hr, ucs], psO_pair[h2][0:HD, :], dnb)

            qproj_emit = None
            outproj_emit = None
            qt_cur = None
            pts_prev = None
            prev_unit = None
            psO_prev = None

            for ui in range(QC * HP):
                qc, hp = divmod(ui, HP)
                if hp == 0:
                    qt_cur = qtg[qc % 2]
                    if qc + 1 < QC:
                        qproj_emit = make_qproj(qc + 1, qtg[(qc + 1) % 2])
                    else:
                        qproj_emit = None
                    outproj_emit = (make_outproj(qc - 1, [(psCp, "psC")])
                                    if qc > 0 else None)

                pts_cur = [[None] * 8 for _ in range(2)]
                if prev_unit is not None:
                    psO_prev = [psOp.tile([128, 512], F32, tag="psO",
                                          name=f"psO{ui}_{h2}") for h2 in range(2)]
                if ui == QC * HP - 1:
                    # final unit: run its own PV in-loop (one iteration behind)
                    psO_self = [psQp.tile([128, 512], F32, tag="psQ", name="psOS0"),
                                psCp.tile([128, 512], F32, tag="psC", name="psOS1")]

                for kp in range(8):
                    it = hp * 8 + kp
                    psS_pair = [psSp.tile([128, 1024], F32, tag="psS",
                                          name=f"psS{ui}_{kp}_{h2}") for h2 in range(2)]
                    # PV for the previous unit first: hides the psS WAR wait
                    # (this iteration's scores reuse the slots the previous
                    # iteration's exps are still draining)
                    if prev_unit is not None:
                        _, php = prev_unit
                        for h2 in range(2):
                            h_prev = php * 2 + h2
                            for u2 in range(2):
                                kk = 2 * kp + u2
                                nc.tensor.matmul(
                                    psO_prev[h2][0:VW, :],
                                    vm[kk][:, h_prev * VW:(h_prev + 1) * VW],
                                    pts_prev[h2][kp][:, u2 * 512:(u2 + 1) * 512],
                                    start=(kk == 0), stop=(kk == KK - 1))
                    # final unit: its own PV one iteration behind
                    if ui == QC * HP - 1 and kp > 0:
                        emit_pv((qc, hp), pts_cur, psO_self, kp - 1)
                    # score matmuls contract over the full 128 partitions;
                    # the zero half of the per-head Q tile nulls the other
                    # head's K rows
                    for u2 in range(2):
                        kk = 2 * kp + u2
                        ks = slice(kk * 128, (kk + 1) * 128)
                        for h2 in range(2):
                            nc.tensor.matmul(psS_pair[h2][:, u2 * 512:(u2 + 1) * 512],
                                             kt[hp][:, ks], qt_cur[hp * 2 + h2],
                                             start=True, stop=True)
                    for h2 in range(2):
                        pt = ptp.tile([128, 1024], BF16, tag="pt",
                                      name=f"pt{ui}_{kp}_{h2}")
                        nc.scalar.activation(pt, psS_pair[h2], EXP, scale=0.125)
                        pts_cur[h2][kp] = pt
                    # unit 0: finish V projection s-tiles 12-15 (needed by
                    # PV(u0) only late in unit 1) to pace PE with the exps
                    if ui == 0:
                        g, half = divmod(kp, 2)
                        st_v = 12 + g
                        ss_v = slice(st_v * 128, (st_v + 1) * 128)
                        if half == 0:
                            vfill_ps = psCp.tile([128, 512], F32, tag="psC",
                                                 name=f"fv{st_v}")
                        for m in range(4):
                            e = half * 4 + m
                            nc.tensor.matmul(vfill_ps, xb[e][:, ss_v], wv_sb[e],
                                             start=(e == 0), stop=(e == ET - 1))
                        if half == 1:
                            emit_vm(st_v, vfill_ps)
                    if qproj_emit is not None:
                        qproj_emit(it)
                    if outproj_emit is not None and 8 <= it < 24:
                        m = (it - 8) * 2
                        outproj_emit(m)
                        outproj_emit(m + 1)
                    if kp == 7 and prev_unit is not None:
                        emit_norms(prev_unit, psO_prev)

                pts_prev = pts_cur
                prev_unit = (qc, hp)

            # epilogue: last PV step + norm for the final unit; the qc=3
            # out-projection spreads over four PSUM slots, pre-running the
            # head-pair 0-2 partials while the final norms drain so the PE
            # stays busy (and at high p-state)
            emit_pv(prev_unit, pts_prev, psO_self, 7)
            emit_norms(prev_unit, psO_self)
            ep_pools = [(psCp, "psC"), (psQp, "psQ"), (psOp, "psO"), (psOp, "psO")]
            ep_ps = {}
            ep_ot = {}

            def ep_mm(grp, hp_i):
                stl, ec = divmod(grp, 2)
                st = (QC - 1) * 4 + stl
                ss = slice(st * 128, (st + 1) * 128)
                es = slice(ec * 512, (ec + 1) * 512)
                if hp_i == 0:
                    pool, ptag = ep_pools[grp % 4]
                    ep_ps[grp] = pool.tile([128, 512], F32, tag=ptag,
                                           name=f"epC{grp}")
                    if ec == 0:
                        ep_ot[stl] = ob.tile([128, D], F32, tag="ot", name=f"ot{st}")
                nc.tensor.matmul(ep_ps[grp], valsT[hp_i][:, ss], wo_sb[hp_i][:, es],
                                 start=(hp_i == 0), stop=(hp_i == HP - 1))
                if hp_i == HP - 1:
                    nc.vector.tensor_copy(ep_ot[stl][:, es], ep_ps[grp])
                    if ec == 1:
                        nc.sync.dma_start(out=out[ss, :], in_=ep_ot[stl])

            for hp_i in range(3):
                for grp in range(4):
                    ep_mm(grp, hp_i)
            for grp in range(4):
                ep_mm(grp, 3)
            for grp in range(4, 8):
                for hp_i in range(HP):
                    ep_mm(grp, hp_i)

    nc.finalize()
    return nc


_NC_CACHE = None


def _get_nc():
    global _NC_CACHE
    if _NC_CACHE is None:
        _NC_CACHE = build_nc()
    return _NC_CACHE


def make_in_maps(x, mask, Wq, bq, Wk, bk, Wv, Wo):
    in_maps = []
    for c in range(8):
        b = c // 2
        dsl = slice((c % 2) * DLOC, (c % 2) * DLOC + DLOC)
        in_maps.append({
            "xT": np.ascontiguousarray(x[b].T, dtype=np.float32),
            "wqT": np.ascontiguousarray(Wq[dsl, :].T, dtype=np.float32),
            "wkT": np.ascontiguousarray(Wk[dsl, :].T, dtype=np.float32),
            "wvT": np.ascontiguousarray(Wv[dsl, :].T, dtype=np.float32),
            "woT": np.ascontiguousarray(Wo[:, dsl].T, dtype=np.float32),
            "bq": np.ascontiguousarray(bq[dsl], dtype=np.float32)[:, None],
            "bk": np.ascontiguousarray(bk[dsl], dtype=np.float32)[:, None],
            "msk": mask[b].astype(np.float32)[:, None],
        })
    return in_maps


def assemble(results, Wo, bo, bv):
    out = np.empty((4, S, D), dtype=np.float32)
    for b in range(4):
        out[b] = results[2 * b]["out"] + results[2 * b + 1]["out"]
    out += (bo + bv @ Wo.T).astype(np.float32)
    return out


def run(x, mask, Wq, bq, Wk, bk, Wv, bv, Wo, bo, trace=False):
    nc = _get_nc()
    in_maps = make_in_maps(x, mask, Wq, bq, Wk, bk, Wv, Wo)
    res = run_bass_kernel_spmd(nc, in_maps, list(range(8)), trace=trace)
    return assemble(res.results, Wo, bo, bv), res


def kernel(x, mask, Wq, bq, Wk, bk, Wv, bv, Wo, bo):
    out, _ = run(x, mask, Wq, bq, Wk, bk, Wv, bv, Wo, bo)
    return out
